# revision 2
# baseline (speedup 1.0000x reference)
"""CapsuleLayer dynamic-routing kernel for TRN2, 8 NeuronCores, batch-sharded.

v2 design (cost-model driven):
  - layout-d: partitions p = (iu,b) [16 i-sub x 8 batch]; u [128, NJ, K, E] bf16.
  - Phase A: per-j creation matmuls (ablk stationary, W moving) fused with
    s0 accumulation matmuls (compact-x stationary) -> v0 ready early; u copied
    PSUM->SBUF on ACT/Pool; iteration-1 agreement (DVE/Pool) hidden under the
    W/ablk DMA stream.
  - Weighted sums via diag-trick matmuls: lhsT = blkdiag(c) built as one 2x
    DVE mul (t_c broadcast x BMASK), fed to PE as a transposed view; the s
    diagonal lands on contiguous (k,b) partitions.
  - Squash on (k,b) partitions; v broadcast back to (iu,b)x(k,e) via constant
    delta_b/kmask matmuls (no partition moves anywhere).
  - No gpsimd bulk ops (slow in cost model); few DMA instructions (HWDGE).
"""
import sys
sys.path.insert(0, "/opt/trn_rl_repo")

import numpy as np
import ml_dtypes

import concourse.bass as bass
import concourse.tile as tile
from concourse import bacc, mybir
from concourse.bass_utils import run_bass_kernel_spmd

NCORES = 8
B, I, K, D, E = 64, 2048, 16, 8, 16
BL = B // NCORES          # 8 batches per core
NJ = I // 16              # 128 blocks of 16 input capsules
EPS = 1e-7

bf16 = mybir.dt.bfloat16
f32 = mybir.dt.float32
FT = mybir.ActivationFunctionType

TRACE = False
_NC_CACHE = {}

WCHUNK = 16               # j's per W DMA chunk
ACHUNK = 32               # j's per ablk DMA chunk
AGCH = 8                  # j's per agreement chunk (16 chunks)


def _bc(ap, shape):
    try:
        return ap.broadcast_to(shape)
    except Exception:
        return ap.to_broadcast(shape)


def _capsule_kernel(tc, vout, ablk, wmv, xc, bmask, delta_b, kmask, onesb8):
    nc = tc.nc
    with (
        tc.tile_pool(name="singles", bufs=1) as singles,
        tc.tile_pool(name="wres", bufs=4) as wres,
        tc.tile_pool(name="ups", bufs=4, space="PSUM") as ups,
        tc.tile_pool(name="s0p", bufs=1, space="PSUM") as s0p,
        tc.tile_pool(name="ssp", bufs=1, space="PSUM") as ssp,
        tc.tile_pool(name="vps", bufs=2, space="PSUM") as vpsp,
        tc.tile_pool(name="ch", bufs=3) as ch,
        tc.tile_pool(name="sm", bufs=2) as sm,
    ):
        # ---- resident tiles ----
        u = singles.tile([128, NJ, K, E], bf16)          # 8 MiB
        ablk_sb = singles.tile([128, NJ, K, BL], bf16)   # 4 MiB; reused as cblk
        xc_sb = singles.tile([128, NJ, BL], bf16)
        bmask_sb = singles.tile([128, K, BL], bf16)
        delta_sb = singles.tile([128, 128], bf16)
        kmask_sb = singles.tile([128, K, E], bf16)
        ones8_sb = singles.tile([8, 128], bf16)
        L = singles.tile([128, NJ, K], bf16)
        ex = singles.tile([128, NJ, K], bf16)
        tc_ = singles.tile([128, NJ, K], bf16)
        cblk = ablk_sb                                    # aliased after phase A
        v_rep = singles.tile([128, K, E], bf16)

        # ---- input DMAs (few, big) ----
        nc.sync.dma_start(out=xc_sb, in_=xc)
        nc.sync.dma_start(out=bmask_sb, in_=bmask)
        nc.sync.dma_start(out=delta_sb, in_=delta_b)
        nc.sync.dma_start(out=kmask_sb, in_=kmask)
        nc.sync.dma_start(out=ones8_sb, in_=onesb8)
        wt = []
        for c0 in range(0, NJ, WCHUNK):
            nc.sync.dma_start(out=ablk_sb[:, c0:c0 + WCHUNK],
                              in_=ablk[:, c0:c0 + WCHUNK])
            wtile = wres.tile([128, WCHUNK, K * E], bf16, tag="wt")
            nc.sync.dma_start(out=wtile,
                              in_=wmv[c0:c0 + WCHUNK].transpose([1, 0, 2]))
            wt.append(wtile)

        # ---- phase A: creation + s0 accumulation ----
        s0_ps = s0p.tile([8, K, E], f32, tag="s0ps")
        copy_engines = []
        for j2 in range(NJ // 2):
            # 2 j's share a psum tile -> bigger copies
            ps2 = ups.tile([128, 2, K, E], f32, tag="ups")
            for t in range(2):
                j = 2 * j2 + t
                wc, wo = j // WCHUNK, j % WCHUNK
                nc.tensor.matmul(ps2[:, t], lhsT=ablk_sb[:, j],
                                 rhs=wt[wc][:, wo], start=True, stop=True,
                                 skip_group_check=True)
                nc.tensor.matmul(s0_ps, lhsT=xc_sb[:, j], rhs=wt[wc][:, wo],
                                 start=(j == 0), stop=(j == NJ - 1),
                                 skip_group_check=True)
            dst = u[:, 2 * j2:2 * j2 + 2]
            r = j2 % 16
            if r < 10:
                nc.scalar.copy(dst, ps2)
            else:
                nc.vector.tensor_copy(dst, ps2)

        # ---- squash0 on [8, K, E] (b partitions) -> v_rep ----
        s0sb = sm.tile([8, K, E], f32, tag="s0sb")
        nc.vector.tensor_copy(s0sb, s0_ps)
        sq0 = sm.tile([8, K, E], f32, tag="sq0")
        nc.vector.tensor_mul(sq0, s0sb, s0sb)
        t8 = sm.tile([8, K, 8], f32, tag="sq8")
        nc.vector.tensor_add(t8, sq0[:, :, 0:8], sq0[:, :, 8:16])
        t4 = sm.tile([8, K, 4], f32, tag="sq4")
        nc.vector.tensor_add(t4, t8[:, :, 0:4], t8[:, :, 4:8])
        t2 = sm.tile([8, K, 2], f32, tag="sq2")
        nc.vector.tensor_add(t2, t4[:, :, 0:2], t4[:, :, 2:4])
        sn = sm.tile([8, K], f32, tag="sn0")
        nc.vector.tensor_add(sn, t2[:, :, 0], t2[:, :, 1])
        sne = sm.tile([8, K], f32, tag="sne0")
        nc.vector.tensor_scalar_add(sne, sn, EPS)
        sqr = sm.tile([8, K], f32, tag="sqr0")
        nc.scalar.activation(sqr, sne, func=FT.Sqrt)
        onep = sm.tile([8, K], f32, tag="onep0")
        nc.vector.tensor_scalar_add(onep, sn, 1.0)
        den = sm.tile([8, K], f32, tag="den0")
        nc.vector.tensor_mul(den, sqr, onep)
        rcp = sm.tile([8, K], f32, tag="rcp0")
        nc.vector.reciprocal(rcp, den)
        fac = sm.tile([8, K], f32, tag="fac0")
        nc.vector.tensor_mul(fac, sn, rcp)
        v0 = sm.tile([8, K, E], bf16, tag="v0")
        nc.vector.tensor_mul(v0, s0sb, _bc(fac.unsqueeze(2), [8, K, E]))
        vr_ps = vpsp.tile([128, K, E], f32, tag="vrps")
        nc.tensor.matmul(vr_ps, lhsT=ones8_sb, rhs=v0, start=True, stop=True,
                         skip_group_check=True)
        nc.vector.tensor_copy(v_rep, vr_ps)

        eps_t = singles.tile([128, 1], f32)
        nc.vector.memset(eps_t, EPS)

        # ---- fused routing iteration: tapered blocks; per block:
        #      agreement chunks -> softmax -> cblk -> s-matmuls ----
        BLOCKS = [(0, 32), (32, 64), (64, 96), (96, 120), (120, 128)]

        def agchunks(b0, b1):
            out = []
            j = b0
            while j < b1:
                w = min(16, b1 - j)
                out.append((j, j + w))
                j += w
            return out

        def full_iter(first, last):
            s_ps = ssp.tile([128, K, E], f32, tag="sps")
            cidx = 0
            hidx = 0
            for (b0, b1) in BLOCKS:
                for (c0, c1) in agchunks(b0, b1):
                    w = c1 - c0
                    jsl = slice(c0, c1)
                    uv = u[:, jsl]
                    vb = _bc(v_rep.unsqueeze(1), [128, w, K, E])
                    on_pool = cidx in (2, 6)
                    cidx += 1
                    prod = ch.tile([128, 16, K, E], bf16, tag="prod")
                    pr = prod[:, 0:w]
                    if on_pool:
                        nc.gpsimd.tensor_mul(pr, uv, vb)
                    else:
                        nc.vector.tensor_mul(pr, uv, vb)
                    padd = nc.gpsimd.tensor_add if on_pool else nc.vector.tensor_add
                    padd(pr[:, :, :, 0:8], pr[:, :, :, 0:8], pr[:, :, :, 8:16])
                    padd(pr[:, :, :, 0:4], pr[:, :, :, 0:4], pr[:, :, :, 4:8])
                    padd(pr[:, :, :, 0:2], pr[:, :, :, 0:2], pr[:, :, :, 2:4])
                    if first:
                        padd(L[:, jsl], pr[:, :, :, 0], pr[:, :, :, 1])
                    else:
                        a1 = ch.tile([128, 16, K], bf16, tag="a1")
                        padd(a1[:, 0:w], pr[:, :, :, 0], pr[:, :, :, 1])
                        padd(L[:, jsl], L[:, jsl], a1[:, 0:w])
                # softmax over k for this block
                bw = b1 - b0
                bsl = slice(b0, b1)
                nc.scalar.activation(ex[:, bsl], L[:, bsl], func=FT.Exp)
                k8 = sm.tile([128, 32, 8], bf16, tag="k8")
                nc.vector.tensor_add(k8[:, 0:bw], ex[:, bsl, 0:8],
                                     ex[:, bsl, 8:16])
                k4 = sm.tile([128, 32, 4], bf16, tag="k4")
                nc.vector.tensor_add(k4[:, 0:bw], k8[:, 0:bw, 0:4],
                                     k8[:, 0:bw, 4:8])
                k2 = sm.tile([128, 32, 2], bf16, tag="k2")
                nc.vector.tensor_add(k2[:, 0:bw], k4[:, 0:bw, 0:2],
                                     k4[:, 0:bw, 2:4])
                ks = sm.tile([128, 32], f32, tag="ks")
                nc.vector.tensor_add(ks[:, 0:bw], k2[:, 0:bw, 0],
                                     k2[:, 0:bw, 1])
                rec = sm.tile([128, 32], f32, tag="rec")
                nc.vector.reciprocal(rec[:, 0:bw], ks[:, 0:bw])
                nc.gpsimd.tensor_mul(
                    tc_[:, bsl], ex[:, bsl],
                    _bc(rec[:, 0:bw].unsqueeze(2), [128, bw, K]))
                # cblk halves then the block's s-matmuls
                h0 = b0
                while h0 < b1:
                    h1 = min(h0 + 16, b1)
                    hsl = slice(h0, h1)
                    hw = h1 - h0
                    tcb = _bc(tc_[:, hsl].unsqueeze(3), [128, hw, K, BL])
                    bmb = _bc(bmask_sb.unsqueeze(1), [128, hw, K, BL])
                    if hidx in (1, 4):
                        nc.gpsimd.tensor_mul(cblk[:, hsl], tcb, bmb)
                    else:
                        nc.vector.tensor_mul(cblk[:, hsl], tcb, bmb)
                    hidx += 1
                    for jj in range(h0, h1):
                        nc.tensor.matmul(
                            s_ps, lhsT=cblk[:, jj],
                            rhs=u[:, jj], start=(jj == 0), stop=(jj == NJ - 1),
                            skip_group_check=True)
                    h0 = h1
            # diagonal extract via kmask + k-tree (full-partition ops only)
            s_sb = sm.tile([128, K, E], f32, tag="s_sb")
            nc.vector.tensor_copy(s_sb, s_ps)
            sDf = sm.tile([128, K, E], f32, tag="sDf")
            nc.vector.tensor_mul(sDf, s_sb, kmask_sb)
            d8 = sm.tile([128, 8, E], f32, tag="d8")
            nc.vector.tensor_add(d8, sDf[:, 0:8], sDf[:, 8:16])
            d4 = sm.tile([128, 4, E], f32, tag="d4")
            nc.vector.tensor_add(d4, d8[:, 0:4], d8[:, 4:8])
            d2 = sm.tile([128, 2, E], f32, tag="d2")
            nc.vector.tensor_add(d2, d4[:, 0:2], d4[:, 2:4])
            sD = sm.tile([128, E], f32, tag="sD")
            nc.vector.tensor_add(sD, d2[:, 0], d2[:, 1])
            # squash on (k,b) partitions
            sq = sm.tile([128, E], f32, tag="sq")
            nc.vector.tensor_mul(sq, sD, sD)
            w8 = sm.tile([128, 8], f32, tag="w8")
            nc.vector.tensor_add(w8, sq[:, 0:8], sq[:, 8:16])
            w4 = sm.tile([128, 4], f32, tag="w4")
            nc.vector.tensor_add(w4, w8[:, 0:4], w8[:, 4:8])
            w2 = sm.tile([128, 2], f32, tag="w2")
            nc.vector.tensor_add(w2, w4[:, 0:2], w4[:, 2:4])
            sn1 = sm.tile([128, 1], f32, tag="sn1")
            nc.vector.tensor_add(sn1, w2[:, 0:1], w2[:, 1:2])
            sne1 = sm.tile([128, 1], f32, tag="sne1")
            nc.vector.tensor_scalar_add(sne1, sn1, EPS)
            sqr1 = sm.tile([128, 1], f32, tag="sqr1")
            nc.scalar.activation(sqr1, sne1, func=FT.Sqrt)
            onep1 = sm.tile([128, 1], f32, tag="onep1")
            nc.vector.tensor_scalar_add(onep1, sn1, 1.0)
            den1 = sm.tile([128, 1], f32, tag="den1")
            nc.vector.tensor_mul(den1, sqr1, onep1)
            rcp1 = sm.tile([128, 1], f32, tag="rcp1")
            nc.vector.reciprocal(rcp1, den1)
            fac1 = sm.tile([128, 1], f32, tag="fac1")
            nc.vector.tensor_mul(fac1, sn1, rcp1)
            if last:
                vf = sm.tile([128, E], f32, tag="vf")
                nc.vector.tensor_mul(vf, sD, _bc(fac1, [128, E]))
                nc.sync.dma_start(out=vout.transpose([1, 0, 2]), in_=vf)
                return
            v_bk = sm.tile([128, E], bf16, tag="v_bk")
            nc.vector.tensor_mul(v_bk, sD, _bc(fac1, [128, E]))
            vbr = sm.tile([128, K, E], bf16, tag="vbr")
            nc.vector.tensor_mul(
                vbr, _bc(v_bk.unsqueeze(1), [128, K, E]), kmask_sb)
            vr2 = vpsp.tile([128, K, E], f32, tag="vrps")
            nc.tensor.matmul(vr2, lhsT=delta_sb, rhs=vbr, start=True,
                             stop=True, skip_group_check=True)
            nc.vector.tensor_copy(v_rep, vr2)

        full_iter(first=True, last=False)
        full_iter(first=False, last=True)


def _build():
    if "nc" in _NC_CACHE:
        return _NC_CACHE["nc"]
    nc = bacc.Bacc("TRN2", target_bir_lowering=False, debug=False,
                   num_devices=NCORES)
    ablk = nc.dram_tensor("ablk", [128, NJ, 128], bf16, kind="ExternalInput").ap()
    wmv = nc.dram_tensor("wmv", [NJ, 128, K * E], bf16, kind="ExternalInput").ap()
    xc = nc.dram_tensor("xc", [128, NJ, BL], bf16, kind="ExternalInput").ap()
    bmask = nc.dram_tensor("bmask", [128, K, BL], bf16, kind="ExternalInput").ap()
    delta_b = nc.dram_tensor("delta_b", [128, 128], bf16, kind="ExternalInput").ap()
    kmask = nc.dram_tensor("kmask", [128, K, E], bf16, kind="ExternalInput").ap()
    onesb8 = nc.dram_tensor("onesb8", [8, 128], bf16, kind="ExternalInput").ap()
    vout = nc.dram_tensor("vout", [BL, K, E], f32, kind="ExternalOutput").ap()
    with tile.TileContext(nc) as tc:
        _capsule_kernel(tc, vout, ablk, wmv, xc, bmask, delta_b, kmask, onesb8)
    nc.compile()
    _NC_CACHE["nc"] = nc
    return nc


def _host_pack(inputs, W):
    # W[i,k,d,e] -> [j, (iu d), (k e)] bf16
    Wb = np.ascontiguousarray(
        W.reshape(NJ, 16, K, D, E).transpose(0, 1, 3, 2, 4)
    ).reshape(NJ, 128, K * E).astype(ml_dtypes.bfloat16)

    # constants
    iu = np.arange(128) // 8
    bq = np.arange(128) % 8
    bmask = np.zeros((128, K, BL), np.float32)
    bmask[np.arange(128), :, bq] = 1.0
    bmask = bmask.astype(ml_dtypes.bfloat16)
    # delta_b: p=(k,b): p = k*8+b ; col=(iu',b') = iu'*8+b' ; 1 if b'==b
    kk = np.arange(128) // 8
    bb = np.arange(128) % 8
    delta_b = np.zeros((128, 128), np.float32)
    for p in range(128):
        delta_b[p, np.arange(16) * 8 + bb[p]] = 1.0
    delta_b = delta_b.astype(ml_dtypes.bfloat16)
    # kmask: p=(k,b) ; [K, E] ; 1 where k'==k
    kmask = np.zeros((128, K, E), np.float32)
    kmask[np.arange(128), kk, :] = 1.0
    kmask = kmask.astype(ml_dtypes.bfloat16)
    # onesb8: [8, 128]: 1 if col%8 == b
    onesb8 = np.zeros((8, 128), np.float32)
    for b in range(8):
        onesb8[b, np.arange(16) * 8 + b] = 1.0
    onesb8 = onesb8.astype(ml_dtypes.bfloat16)

    in_maps = []
    for c in range(NCORES):
        inp_c = inputs[c * BL:(c + 1) * BL]          # [8, 2048, 8]
        inp_t = inp_c.reshape(BL, NJ, 16, D)         # b, j, iu, d
        ab = np.zeros((16, D, NJ, 16, BL), np.float32)
        for g in range(16):
            ab[g, :, :, g, :] = inp_t[:, :, g, :].transpose(2, 1, 0)
        ab = ab.reshape(128, NJ, 128).astype(ml_dtypes.bfloat16)
        # xc[(iu,d), j, b] = x[b, (j,iu), d] / 16
        xcv = (inp_t.transpose(2, 3, 1, 0) / 16.0)   # iu, d, j, b
        xcv = xcv.reshape(128, NJ, BL).astype(ml_dtypes.bfloat16)
        in_maps.append({"ablk": ab, "wmv": Wb, "xc": xcv, "bmask": bmask,
                        "delta_b": delta_b, "kmask": kmask, "onesb8": onesb8})
    return in_maps


def kernel(inputs, W):
    inputs = np.asarray(inputs, np.float32)
    W = np.asarray(W, np.float32)
    nc = _build()
    in_maps = _host_pack(inputs, W)
    br = run_bass_kernel_spmd(nc, in_maps, core_ids=list(range(NCORES)),
                              trace=TRACE)
    if br.exec_time_ns is not None:
        print(f"HW exec time: {br.exec_time_ns} ns")
    # vout [BL, K, E] per core; p=(k,b) diag already reordered by the DMA AP
    out = np.concatenate([r["vout"] for r in br.results], axis=0)
    return out.astype(np.float32)
